# revision 6
# baseline (speedup 1.0000x reference)
"""Trainium2 Bass kernel for GQA attention (nn_Attention_75539884802796).

Sharding: data-parallel over batch — B=8 batch elements across 8 NeuronCores,
one full attention layer per core, zero collectives.

Per-core pipeline (static/unrolled, Tile-scheduled):
  1. q/k/v projections as f32r matmuls (xT resident, weights streamed once)
  2. RoPE applied in natural [token, head*dim] layout on DVE (reads PSUM)
  3. PE transposes q,k -> qT,kT for the scores matmul
  4. per head: scores = q @ kT (causal-width only), diag-mask add,
     exp with fused row-sum on ACT, reciprocal+normalize, PE transpose of
     probs, AV matmul accumulated per head pair; o_proj at the end
Outputs: out [S, HIDDEN] f32, k/v caches [HKV, S, D] f32 per batch.

Self-contained: hardcodes all shapes; no sibling imports.

Toolchain compatibility patches (this image's walrus):
  - sync waits are limited to 1 per instruction (0 for Drain/NoOp); Tile
    fuses many waits onto one instruction -> split them into standalone
    EventSemaphore instructions at BIR-JSON serialization time.
  - Tile's tail drain+barrier emits Drains carrying sync -> replaced with
    single-wait instructions and sem-only barriers.
"""

import numpy as np
import orjson

import concourse.bass as bass
import concourse.mybir as mybir
from concourse.bass_utils import run_bass_kernel_spmd
from concourse.tile import TileContext
from concourse.vector_clock import ScopedClock

HIDDEN = 2048
N_HEADS = 32
N_KV_HEADS = 8
HEAD_DIM = 64
S = 512
B = 8
ROPE_BASE = 10000.0

KT = HIDDEN // 128        # 16 contraction tiles
TT = S // 128             # 4 token tiles
QCH = N_HEADS * HEAD_DIM // 512   # 4 q-projection column chunks (8 heads each)
GROUPS = N_HEADS // N_KV_HEADS

F32 = mybir.dt.float32
F32R = mybir.dt.float32r
AX = mybir.AluOpType


# --------------------------------------------------------------------------
# toolchain compatibility patches
# --------------------------------------------------------------------------

def _patch_tile_tail():
    if getattr(TileContext, "_tail_patched", False):
        return

    def patched(self, tick_clock, wait_clock):
        nc = self.nc
        tmp = nc.sync.nop(nofuse=True)
        wait_clock.add_sem_waits(tmp.ins, ScopedClock({None: tick_clock.global_clock}))
        waits = list(tmp.ins.sync_info.on_wait)
        del tmp.ins.sync_info.on_wait[:]
        id2sem = {sem.num: sem for sem in self.sems.allocated().values()}
        for w in waits:
            sem = id2sem.get(w.id)
            assert sem is not None, f"unknown sem id {w.id}"
            nc.sync.wait_ge(sem, w.wait_value)
        nc.all_engine_barrier(sem_only=True)
        popped = nc._tile_sem_poison_stack.pop()
        assert popped is self._sem_poison
        nc.clear_and_free_semaphores(list(self.sems.allocated().values()))
        nc.all_engine_barrier(sem_only=True)

    TileContext._drain_and_barrier = patched
    TileContext._tail_patched = True


def _split_multi_waits(bir: dict) -> dict:
    """Walrus here accepts at most one sync wait per instruction (none on
    Drain/NoOp). Hoist extra waits onto standalone EventSemaphore
    instructions inserted just before, on the same engine."""
    n_new = 0
    for fn in bir.get("functions", []):
        for blk in fn.get("blocks", []):
            insts = blk.get("instructions")
            if not insts:
                continue
            out = []
            for inst in insts:
                si = inst.get("sync_info")
                waits = (si or {}).get("on_wait") or []
                keep = 0 if inst.get("opcode") in ("Drain", "NoOp") else 1
                if len(waits) > keep:
                    split = waits[: len(waits) - keep]
                    si["on_wait"] = waits[len(waits) - keep:]
                    for w in split:
                        n_new += 1
                        out.append({
                            "debug": inst.get("debug", {}),
                            "engine": inst["engine"],
                            "ins": [],
                            "name": f"{inst['name']}_sw{n_new}",
                            "opcode": "EventSemaphore",
                            "outs": [],
                            "sync_info": {"on_update": [], "on_wait": [w]},
                        })
                out.append(inst)
            blk["instructions"] = out
    return bir


def _patch_to_json():
    if getattr(bass.Bass, "_json_multiwait_patched", False):
        return
    orig = bass.Bass.to_json_bytes

    def patched(self):
        data = orig(self)
        bir = orjson.loads(data)
        bir = _split_multi_waits(bir)
        return orjson.dumps(bir)

    bass.Bass.to_json_bytes = patched
    bass.Bass._json_multiwait_patched = True


def apply_patches():
    _patch_tile_tail()
    _patch_to_json()


# --------------------------------------------------------------------------
# kernel graph
# --------------------------------------------------------------------------

def build_nc():
    """Build the per-core Bass graph (same graph on all 8 cores)."""
    apply_patches()
    nc = bass.Bass("TRN2", target_bir_lowering=False)

    xt_d = nc.dram_tensor("xt", [HIDDEN, S], F32R, kind="ExternalInput")
    wq_d = nc.dram_tensor("wq", [HIDDEN, N_HEADS * HEAD_DIM], F32R, kind="ExternalInput")
    wk_d = nc.dram_tensor("wk", [HIDDEN, N_KV_HEADS * HEAD_DIM], F32R, kind="ExternalInput")
    wv_d = nc.dram_tensor("wv", [HIDDEN, N_KV_HEADS * HEAD_DIM], F32R, kind="ExternalInput")
    wo_d = nc.dram_tensor("wo", [N_HEADS * HEAD_DIM, HIDDEN], F32R, kind="ExternalInput")
    cos_d = nc.dram_tensor("cos8", [S, 512], F32, kind="ExternalInput")
    sin_d = nc.dram_tensor("sin8", [S, 512], F32, kind="ExternalInput")
    id_d = nc.dram_tensor("ident", [128, 128], F32R, kind="ExternalInput")
    dm_d = nc.dram_tensor("dmask", [128, 128], F32, kind="ExternalInput")

    out_d = nc.dram_tensor("out", [S, HIDDEN], F32, kind="ExternalOutput")
    kc_d = nc.dram_tensor("kc", [N_KV_HEADS, S, HEAD_DIM], F32, kind="ExternalOutput")
    vc_d = nc.dram_tensor("vc", [N_KV_HEADS, S, HEAD_DIM], F32, kind="ExternalOutput")

    with TileContext(nc) as tc:
        with (
            tc.tile_pool(name="const", bufs=1) as cpool,
            tc.tile_pool(name="resident", bufs=1) as rpool,
            tc.tile_pool(name="wstream", bufs=12) as wpool,
            tc.tile_pool(name="work", bufs=3) as work,
            tc.tile_pool(name="probs_sb", bufs=3) as probs_pool,
            tc.tile_pool(name="probsT_sb", bufs=6) as pT_pool,
            tc.tile_pool(name="stats", bufs=8) as stat,
        ):
            # ---- constants / resident tensors ----
            ident = cpool.tile([128, 128], F32R, name="ident")
            nc.sync.dma_start(ident[:], id_d[:])
            dmask = cpool.tile([128, 128], F32, name="dmask")
            nc.sync.dma_start(dmask[:], dm_d[:])
            cos8 = cpool.tile([128, TT, 512], F32, name="cos8")
            nc.sync.dma_start(cos8[:], cos_d.rearrange("(t p) n -> p t n", p=128))
            sin8 = cpool.tile([128, TT, 512], F32, name="sin8")
            nc.sync.dma_start(sin8[:], sin_d.rearrange("(t p) n -> p t n", p=128))

            xt = rpool.tile([128, KT, S], F32R, name="xt")
            nc.sync.dma_start(xt[:], xt_d.rearrange("(t p) m -> p t m", p=128))

            qT = rpool.tile([128, N_HEADS // 2, S], F32R, name="qT")
            kTd = rpool.tile([128, N_KV_HEADS, S], F32R, name="kTd")  # dup halves
            v_sb = rpool.tile([128, TT, 512], F32R, name="v_sb")
            aoT = rpool.tile([128, KT, S], F32R, name="aoT")

            def rope_block(ps_in, cos_t, sin_t, out_tile):
                """RoPE on a [128 tok, 512 = 8 heads x 64] natural block.
                ps_in: PSUM f32 AP; cos_t/sin_t: [128, 512] sbuf f32 APs;
                out_tile: [128, 512] sbuf F32 tile."""
                qc = work.tile([128, 512], F32, name="ropeqc", tag="ropeqc")
                nc.vector.tensor_tensor(qc[:], ps_in, cos_t, op=AX.mult)
                ps3 = ps_in.rearrange("p (g c) -> p g c", c=64)
                qc3 = qc.rearrange("p (g c) -> p g c", c=64)
                sn3 = sin_t.rearrange("p (g c) -> p g c", c=64)
                ot3 = out_tile.rearrange("p (g c) -> p g c", c=64)
                t1 = work.tile([128, 256], F32, name="ropet1", tag="ropet1")
                t13 = t1.rearrange("p (g c) -> p g c", c=32)
                nc.vector.tensor_tensor(t13[:], ps3[:, :, 32:64], sn3[:, :, 0:32], op=AX.mult)
                nc.vector.tensor_tensor(ot3[:, :, 0:32], qc3[:, :, 0:32], t13[:], op=AX.subtract)
                t2 = work.tile([128, 256], F32, name="ropet2", tag="ropet2")
                t23 = t2.rearrange("p (g c) -> p g c", c=32)
                nc.vector.tensor_tensor(t23[:], ps3[:, :, 0:32], sn3[:, :, 32:64], op=AX.mult)
                nc.vector.tensor_tensor(ot3[:, :, 32:64], qc3[:, :, 32:64], t23[:], op=AX.add)

            # ================= q/k/v projections, rope, transposes ===========
            with (
                tc.tile_pool(name="psP", bufs=1, space="PSUM") as psP,
                tc.tile_pool(name="psT1", bufs=2, space="PSUM") as psT1,
            ):
                for o in range(QCH):
                    wq_t = []
                    for kt in range(KT):
                        w = wpool.tile([128, 512], F32R, name=f"wq_{o}_{kt}", tag="wtile")
                        nc.sync.dma_start(
                            w[:], wq_d[kt * 128:(kt + 1) * 128, o * 512:(o + 1) * 512]
                        )
                        wq_t.append(w)
                    ps_q = [psP.tile([128, 512], F32, name=f"ps_q_{o}_{t}", tag=f"ps_q{t}")
                            for t in range(TT)]
                    for kt in range(KT):
                        for t in range(TT):
                            nc.tensor.matmul(
                                ps_q[t][:], xt[:, kt, t * 128:(t + 1) * 128], wq_t[kt][:],
                                start=(kt == 0), stop=(kt == KT - 1),
                            )
                    for t in range(TT):
                        qrot = work.tile([128, 512], F32R, name="qrot", tag="rot")
                        rope_block(ps_q[t][:], cos8[:, t, :], sin8[:, t, :], qrot)
                        ps_tr = psT1.tile([128, 512], F32R, name=f"ps_trq_{o}_{t}", tag="ps_tr")
                        for sub in range(4):
                            nc.tensor.transpose(
                                ps_tr[:, sub * 128:(sub + 1) * 128],
                                qrot[:, sub * 128:(sub + 1) * 128],
                                ident[:],
                            )
                        nc.vector.tensor_copy(
                            qT[:, 4 * o:4 * o + 4, t * 128:(t + 1) * 128],
                            ps_tr.rearrange("p (a b) -> p a b", b=128),
                        )

                # ---- k ----
                wk_t = []
                for kt in range(KT):
                    w = wpool.tile([128, 512], F32R, name=f"wk_{kt}", tag="wtile")
                    nc.sync.dma_start(w[:], wk_d[kt * 128:(kt + 1) * 128, :])
                    wk_t.append(w)
                ps_k = [psP.tile([128, 512], F32, name=f"ps_k_{t}", tag=f"ps_q{t}")
                        for t in range(TT)]
                for kt in range(KT):
                    for t in range(TT):
                        nc.tensor.matmul(
                            ps_k[t][:], xt[:, kt, t * 128:(t + 1) * 128], wk_t[kt][:],
                            start=(kt == 0), stop=(kt == KT - 1),
                        )
                kc_r = kc_d.rearrange("g s d -> s g d")
                for t in range(TT):
                    krot = work.tile([128, 512], F32R, name="krot", tag="rot")
                    rope_block(ps_k[t][:], cos8[:, t, :], sin8[:, t, :], krot)
                    nc.sync.dma_start(
                        kc_r[t * 128:(t + 1) * 128, :, :],
                        krot.bitcast(F32).rearrange("p (g d) -> p g d", d=64),
                    )
                    ps_tr = psT1.tile([128, 512], F32R, name=f"ps_trk_{t}", tag="ps_tr")
                    for sub in range(4):
                        nc.tensor.transpose(
                            ps_tr[:, sub * 128:(sub + 1) * 128],
                            krot[:, sub * 128:(sub + 1) * 128],
                            ident[:],
                        )
                    for sub in range(4):
                        for half in range(2):
                            g = 2 * sub + half
                            src = ps_tr[64 * half:64 * half + 64,
                                        sub * 128:(sub + 1) * 128]
                            nc.vector.tensor_copy(
                                kTd[0:64, g, t * 128:(t + 1) * 128], src)
                            nc.vector.tensor_copy(
                                kTd[64:128, g, t * 128:(t + 1) * 128], src)

                # ---- v ----
                wv_t = []
                for kt in range(KT):
                    w = wpool.tile([128, 512], F32R, name=f"wv_{kt}", tag="wtile")
                    nc.sync.dma_start(w[:], wv_d[kt * 128:(kt + 1) * 128, :])
                    wv_t.append(w)
                ps_v = [psP.tile([128, 512], F32, name=f"ps_v_{t}", tag=f"ps_q{t}")
                        for t in range(TT)]
                for kt in range(KT):
                    for t in range(TT):
                        nc.tensor.matmul(
                            ps_v[t][:], xt[:, kt, t * 128:(t + 1) * 128], wv_t[kt][:],
                            start=(kt == 0), stop=(kt == KT - 1),
                        )
                vc_r = vc_d.rearrange("g s d -> s g d")
                for t in range(TT):
                    nc.vector.tensor_copy(v_sb[:, t, :], ps_v[t][:])
                    nc.sync.dma_start(
                        vc_r[t * 128:(t + 1) * 128, :, :],
                        v_sb.bitcast(F32)[:, t, :].rearrange("p (g d) -> p g d", d=64),
                    )

            # ================= attention per head =============================
            with (
                tc.tile_pool(name="psS", bufs=2, space="PSUM") as psS,
                tc.tile_pool(name="psPT", bufs=1, space="PSUM") as psPT,
                tc.tile_pool(name="psO", bufs=2, space="PSUM") as psO,
            ):
                for h in range(N_HEADS):
                    g = h // GROUPS
                    hp = h // 2
                    ho = (h % 2) * 64
                    psT = [psPT.tile([128, 512], F32R, name=f"psT_{h}_{j}", tag=f"psT{j}")
                           for j in range(TT)]
                    for i in range(TT):
                        wdt = (i + 1) * 128
                        ps_s = psS.tile([128, 512], F32, name=f"ps_s_{h}_{i}", tag="ps_s")
                        nc.tensor.matmul(
                            ps_s[:, :wdt],
                            qT[ho:ho + 64, hp, i * 128:(i + 1) * 128],
                            kTd[ho:ho + 64, g, 0:wdt],
                            start=True, stop=True,
                        )
                        nc.vector.tensor_tensor(
                            ps_s[:, wdt - 128:wdt], ps_s[:, wdt - 128:wdt], dmask[:],
                            op=AX.add,
                        )
                        probs = probs_pool.tile([128, 512], F32R, name=f"probs_{h}_{i}",
                                                tag="probs")
                        sums = stat.tile([128, 1], F32, name=f"sums_{h}_{i}", tag="sums")
                        nc.scalar.activation(
                            probs[:, :wdt], ps_s[:, :wdt],
                            mybir.ActivationFunctionType.Exp,
                            accum_out=sums[:],
                        )
                        rec = stat.tile([128, 1], F32, name=f"rec_{h}_{i}", tag="rec")
                        nc.vector.reciprocal(rec[:], sums[:])
                        nc.vector.tensor_scalar_mul(probs[:, :wdt], probs[:, :wdt], rec[:])
                        for j in range(i + 1):
                            nc.tensor.transpose(
                                psT[j][:, i * 128:(i + 1) * 128],
                                probs[:, j * 128:(j + 1) * 128],
                                ident[:],
                            )
                    pT_sb = []
                    for j in range(TT):
                        pt = pT_pool.tile([128, 512], F32R, name=f"pT_{h}_{j}", tag="pT")
                        nc.vector.tensor_copy(pt[:, j * 128:], psT[j][:, j * 128:])
                        pT_sb.append(pt)
                    ps_o = psO.tile([64, 512], F32, name=f"ps_o_{h}", tag="ps_o")
                    for j in range(TT):
                        nc.tensor.matmul(
                            ps_o[:, j * 128:],
                            v_sb[:, j, g * 64:(g + 1) * 64],
                            pT_sb[j][:, j * 128:],
                            start=(j == 0), stop=(j == TT - 1),
                        )
                    nc.vector.tensor_copy(aoT[ho:ho + 64, hp, :], ps_o[:])

            # ================= o_proj =========================================
            with tc.tile_pool(name="psF", bufs=1, space="PSUM") as psF:
                for o in range(4):
                    wo_t = []
                    for kt in range(KT):
                        w = wpool.tile([128, 512], F32R, name=f"wo_{o}_{kt}", tag="wtile")
                        nc.sync.dma_start(
                            w[:], wo_d[kt * 128:(kt + 1) * 128, o * 512:(o + 1) * 512]
                        )
                        wo_t.append(w)
                    ps_out = [psF.tile([128, 512], F32, name=f"ps_out_{o}_{t}",
                                       tag=f"ps_out{t}") for t in range(TT)]
                    for kt in range(KT):
                        for t in range(TT):
                            nc.tensor.matmul(
                                ps_out[t][:], aoT[:, kt, t * 128:(t + 1) * 128],
                                wo_t[kt][:],
                                start=(kt == 0), stop=(kt == KT - 1),
                            )
                    for t in range(TT):
                        ot = work.tile([128, 512], F32, name="out_sb", tag="out_sb")
                        nc.vector.tensor_copy(ot[:], ps_out[t][:])
                        nc.sync.dma_start(
                            out_d[t * 128:(t + 1) * 128, o * 512:(o + 1) * 512], ot[:]
                        )

    return nc


_nc_cache = [None]


def _rope_tables():
    inv_freq = 1.0 / (ROPE_BASE ** (np.arange(0, HEAD_DIM, 2, dtype=np.float32) / HEAD_DIM))
    pos = np.arange(S, dtype=np.float32)
    freqs = np.outer(pos, inv_freq)
    emb = np.concatenate([freqs, freqs], axis=-1)  # [S, D]
    return np.cos(emb).astype(np.float32), np.sin(emb).astype(np.float32)


def prepare_in_maps(x, Wq, Wk, Wv, Wo):
    scale = np.float32(HEAD_DIM ** -0.5)
    cos, sin = _rope_tables()
    cos8 = np.ascontiguousarray(np.tile(cos, (1, N_KV_HEADS)))   # [S, 512]
    sin8 = np.ascontiguousarray(np.tile(sin, (1, N_KV_HEADS)))
    ident = np.eye(128, dtype=np.float32)
    dmask = np.triu(np.full((128, 128), -1e30, dtype=np.float32), k=1)
    wq_s = np.ascontiguousarray(Wq.astype(np.float32) * scale)
    wk = np.ascontiguousarray(Wk.astype(np.float32))
    wv = np.ascontiguousarray(Wv.astype(np.float32))
    wo = np.ascontiguousarray(Wo.astype(np.float32))
    in_maps = []
    for b in range(B):
        in_maps.append({
            "xt": np.ascontiguousarray(x[b].T.astype(np.float32)),
            "wq": wq_s, "wk": wk, "wv": wv, "wo": wo,
            "cos8": cos8, "sin8": sin8, "ident": ident, "dmask": dmask,
        })
    return in_maps


def run(x, Wq, Wk, Wv, Wo, trace=False, **spmd_kwargs):
    if _nc_cache[0] is None:
        _nc_cache[0] = build_nc()
    nc = _nc_cache[0]
    in_maps = prepare_in_maps(x, Wq, Wk, Wv, Wo)
    res = run_bass_kernel_spmd(nc, in_maps, core_ids=list(range(B)), trace=trace,
                               **spmd_kwargs)
    out = np.stack([res.results[b]["out"] for b in range(B)])       # [B, S, H]
    kc = np.stack([res.results[b]["kc"] for b in range(B)])         # [B, Hkv, S, D]
    vc = np.stack([res.results[b]["vc"] for b in range(B)])
    return (out, kc, vc), res


def kernel(x, Wq, Wk, Wv, Wo):
    (out, kc, vc), _ = run(np.asarray(x), np.asarray(Wq), np.asarray(Wk),
                           np.asarray(Wv), np.asarray(Wo), trace=False)
    return out.astype(np.float32), kc.astype(np.float32), vc.astype(np.float32)


# revision 11
# speedup vs baseline: 1.1065x; 1.1065x over previous
"""Trainium2 Bass kernel for GQA attention (nn_Attention_75539884802796).

Sharding: data-parallel over batch — B=8 batch elements across 8 NeuronCores,
one full attention layer per core, zero collectives.

Per-core pipeline (static/unrolled, Tile-scheduled, bf16 matmul path):
  1. q/k/v projections as bf16 matmuls (xT resident, weights streamed once)
  2. RoPE applied in natural [token, head*dim] layout on DVE (reads PSUM f32)
  3. PE transposes q,k -> qT,kT (kT duplicated into both partition halves)
  4. per head-pair: scores = q @ kT issued back-to-back for both heads
     (disjoint PE row groups overlap), diag-mask add, exp with fused row-sum
     on ACT, reciprocal+normalize, PE transpose of probs, AV matmul
  5. o_proj from the transposed attention output
Outputs: out [S, HIDDEN] f32, k/v caches [HKV, S, D] f32 per batch.

Self-contained: hardcodes all shapes; no sibling imports.

Toolchain compatibility patches (this image's walrus):
  - sync waits are limited to 1 per instruction (0 for Drain/NoOp); Tile
    fuses many waits onto one instruction -> split them into standalone
    EventSemaphore instructions at BIR-JSON serialization time.
  - Tile's tail drain+barrier emits Drains carrying sync -> replaced with
    single-wait instructions and sem-only barriers.
"""

import ml_dtypes
import numpy as np
import orjson

import concourse.bass as bass
import concourse.mybir as mybir
from concourse.bass_utils import run_bass_kernel_spmd
from concourse.tile import TileContext
from concourse.vector_clock import ScopedClock

HIDDEN = 2048
N_HEADS = 32
N_KV_HEADS = 8
HEAD_DIM = 64
S = 512
B = 8
ROPE_BASE = 10000.0

KT = HIDDEN // 128        # 16 contraction tiles
TT = S // 128             # 4 token tiles
QCH = N_HEADS * HEAD_DIM // 512   # 4 q-projection column chunks (8 heads each)
GROUPS = N_HEADS // N_KV_HEADS

F32 = mybir.dt.float32
BF16 = mybir.dt.bfloat16
AX = mybir.AluOpType
BF = ml_dtypes.bfloat16


# --------------------------------------------------------------------------
# toolchain compatibility patches
# --------------------------------------------------------------------------

def _patch_tile_tail():
    if getattr(TileContext, "_tail_patched", False):
        return

    def patched(self, tick_clock, wait_clock):
        nc = self.nc
        tmp = nc.sync.nop(nofuse=True)
        wait_clock.add_sem_waits(tmp.ins, ScopedClock({None: tick_clock.global_clock}))
        waits = list(tmp.ins.sync_info.on_wait)
        del tmp.ins.sync_info.on_wait[:]
        id2sem = {sem.num: sem for sem in self.sems.allocated().values()}
        for w in waits:
            sem = id2sem.get(w.id)
            assert sem is not None, f"unknown sem id {w.id}"
            nc.sync.wait_ge(sem, w.wait_value)
        nc.all_engine_barrier(sem_only=True)
        popped = nc._tile_sem_poison_stack.pop()
        assert popped is self._sem_poison
        nc.clear_and_free_semaphores(list(self.sems.allocated().values()))
        nc.all_engine_barrier(sem_only=True)

    TileContext._drain_and_barrier = patched
    TileContext._tail_patched = True


def _split_multi_waits(bir: dict) -> dict:
    """Walrus here accepts at most one sync wait per instruction (none on
    Drain/NoOp). Hoist extra waits onto standalone EventSemaphore
    instructions inserted just before, on the same engine."""
    n_new = 0
    for fn in bir.get("functions", []):
        for blk in fn.get("blocks", []):
            insts = blk.get("instructions")
            if not insts:
                continue
            out = []
            for inst in insts:
                si = inst.get("sync_info")
                waits = (si or {}).get("on_wait") or []
                keep = 0 if inst.get("opcode") in ("Drain", "NoOp") else 1
                if len(waits) > keep:
                    split = waits[: len(waits) - keep]
                    si["on_wait"] = waits[len(waits) - keep:]
                    for w in split:
                        n_new += 1
                        out.append({
                            "debug": inst.get("debug", {}),
                            "engine": inst["engine"],
                            "ins": [],
                            "name": f"{inst['name']}_sw{n_new}",
                            "opcode": "EventSemaphore",
                            "outs": [],
                            "sync_info": {"on_update": [], "on_wait": [w]},
                        })
                out.append(inst)
            blk["instructions"] = out
    return bir


def _patch_to_json():
    if getattr(bass.Bass, "_json_multiwait_patched", False):
        return
    orig = bass.Bass.to_json_bytes

    def patched(self):
        data = orig(self)
        bir = orjson.loads(data)
        bir = _split_multi_waits(bir)
        return orjson.dumps(bir)

    bass.Bass.to_json_bytes = patched
    bass.Bass._json_multiwait_patched = True


def apply_patches():
    _patch_tile_tail()
    _patch_to_json()


# --------------------------------------------------------------------------
# kernel graph
# --------------------------------------------------------------------------

def build_nc():
    """Build the per-core Bass graph (same graph on all 8 cores)."""
    apply_patches()
    nc = bass.Bass("TRN2", target_bir_lowering=False)

    xt_d = nc.dram_tensor("xt", [HIDDEN, S], BF16, kind="ExternalInput")
    wq_d = nc.dram_tensor("wq", [HIDDEN, N_HEADS * HEAD_DIM], BF16, kind="ExternalInput")
    wk_d = nc.dram_tensor("wk", [HIDDEN, N_KV_HEADS * HEAD_DIM], BF16, kind="ExternalInput")
    wv_d = nc.dram_tensor("wv", [HIDDEN, N_KV_HEADS * HEAD_DIM], BF16, kind="ExternalInput")
    wo_d = nc.dram_tensor("wo", [N_HEADS * HEAD_DIM, HIDDEN], BF16, kind="ExternalInput")
    cos_d = nc.dram_tensor("cos8", [S, 512], F32, kind="ExternalInput")
    sin_d = nc.dram_tensor("sin8", [S, 512], F32, kind="ExternalInput")
    id_d = nc.dram_tensor("ident", [128, 128], BF16, kind="ExternalInput")
    dm_d = nc.dram_tensor("dmask", [128, 128], F32, kind="ExternalInput")

    out_d = nc.dram_tensor("out", [S, HIDDEN], F32, kind="ExternalOutput")
    kc_d = nc.dram_tensor("kc", [N_KV_HEADS, S, HEAD_DIM], F32, kind="ExternalOutput")
    vc_d = nc.dram_tensor("vc", [N_KV_HEADS, S, HEAD_DIM], F32, kind="ExternalOutput")

    with TileContext(nc) as tc:
        with (
            tc.tile_pool(name="const", bufs=1) as cpool,
            tc.tile_pool(name="resident", bufs=1) as rpool,
            tc.tile_pool(name="wstream", bufs=3) as wpool,
            tc.tile_pool(name="work", bufs=3) as work,
            tc.tile_pool(name="probs_sb", bufs=4) as probs_pool,
            tc.tile_pool(name="probsT_sb", bufs=8) as pT_pool,
            tc.tile_pool(name="stats", bufs=8) as stat,
        ):
            # ---- constants / resident tensors ----
            ident = cpool.tile([128, 128], BF16, name="ident")
            nc.sync.dma_start(ident[:], id_d[:])
            dmask = cpool.tile([128, 128], F32, name="dmask")
            nc.sync.dma_start(dmask[:], dm_d[:])
            cos8 = cpool.tile([128, TT, 512], F32, name="cos8")
            nc.sync.dma_start(cos8[:], cos_d.rearrange("(t p) n -> p t n", p=128))
            sin8 = cpool.tile([128, TT, 512], F32, name="sin8")
            nc.sync.dma_start(sin8[:], sin_d.rearrange("(t p) n -> p t n", p=128))

            xt = rpool.tile([128, KT, S], BF16, name="xt")
            nc.sync.dma_start(xt[:], xt_d.rearrange("(t p) m -> p t m", p=128))

            qT = rpool.tile([64, N_HEADS, S], BF16, name="qT")
            kT = rpool.tile([64, N_KV_HEADS, S], BF16, name="kT")
            v_sb = rpool.tile([128, TT, 512], BF16, name="v_sb")
            aoT = rpool.tile([128, KT, S], BF16, name="aoT")

            def rope_block(ps_in, cos_t, sin_t, out_tile):
                """RoPE on a [128 tok, 512 = 8 heads x 64] natural block.
                ps_in: PSUM f32 AP; cos_t/sin_t: [128, 512] sbuf f32 APs;
                out_tile: [128, 512] sbuf BF16 tile."""
                qc = work.tile([128, 512], F32, name="ropeqc", tag="ropeqc")
                nc.vector.tensor_tensor(qc[:], ps_in, cos_t, op=AX.mult)
                ps3 = ps_in.rearrange("p (g c) -> p g c", c=64)
                qc3 = qc.rearrange("p (g c) -> p g c", c=64)
                sn3 = sin_t.rearrange("p (g c) -> p g c", c=64)
                ot3 = out_tile.rearrange("p (g c) -> p g c", c=64)
                t1 = work.tile([128, 256], F32, name="ropet1", tag="ropet1")
                t13 = t1.rearrange("p (g c) -> p g c", c=32)
                nc.vector.tensor_tensor(t13[:], ps3[:, :, 32:64], sn3[:, :, 0:32], op=AX.mult)
                nc.vector.tensor_tensor(ot3[:, :, 0:32], qc3[:, :, 0:32], t13[:], op=AX.subtract)
                t2 = work.tile([128, 256], F32, name="ropet2", tag="ropet2")
                t23 = t2.rearrange("p (g c) -> p g c", c=32)
                nc.vector.tensor_tensor(t23[:], ps3[:, :, 0:32], sn3[:, :, 32:64], op=AX.mult)
                nc.vector.tensor_tensor(ot3[:, :, 32:64], qc3[:, :, 32:64], t23[:], op=AX.add)

            # ================= q/k/v projections, rope, transposes ===========
            with (
                tc.tile_pool(name="psP", bufs=1, space="PSUM") as psP,
                tc.tile_pool(name="psT1", bufs=2, space="PSUM") as psT1,
            ):
                for o in range(QCH):
                    wq_t = wpool.tile([128, KT, 512], BF16, name=f"wq_{o}", tag="wtile")
                    nc.sync.dma_start(
                        wq_t[:],
                        wq_d[:, o * 512:(o + 1) * 512].rearrange("(t p) n -> p t n", p=128),
                    )
                    ps_q = [psP.tile([128, 512], F32, name=f"ps_q_{o}_{t}", tag=f"ps_q{t}")
                            for t in range(TT)]
                    for kt in range(KT):
                        for t in range(TT):
                            nc.tensor.matmul(
                                ps_q[t][:], xt[:, kt, t * 128:(t + 1) * 128],
                                wq_t[:, kt, :],
                                start=(kt == 0), stop=(kt == KT - 1),
                            )
                    for t in range(TT):
                        qrot = work.tile([128, 512], BF16, name="qrot", tag="rot")
                        rope_block(ps_q[t][:], cos8[:, t, :], sin8[:, t, :], qrot)
                        ps_tr = psT1.tile([64, 8, 128], BF16, name=f"ps_trq_{o}_{t}", tag="ps_tr")
                        for hh in range(8):
                            nc.tensor.transpose(
                                ps_tr[:, hh, :],
                                qrot[:, hh * 64:(hh + 1) * 64],
                                ident[:],
                            )
                        nc.vector.tensor_copy(
                            qT[:, 8 * o:8 * o + 8, t * 128:(t + 1) * 128], ps_tr[:]
                        )

                # ---- k ----
                wk_t = wpool.tile([128, KT, 512], BF16, name="wk", tag="wtile")
                nc.sync.dma_start(wk_t[:], wk_d.rearrange("(t p) n -> p t n", p=128))
                ps_k = [psP.tile([128, 512], F32, name=f"ps_k_{t}", tag=f"ps_q{t}")
                        for t in range(TT)]
                for kt in range(KT):
                    for t in range(TT):
                        nc.tensor.matmul(
                            ps_k[t][:], xt[:, kt, t * 128:(t + 1) * 128], wk_t[:, kt, :],
                            start=(kt == 0), stop=(kt == KT - 1),
                        )
                kc_r = kc_d.rearrange("g s d -> s g d")
                for t in range(TT):
                    krot_f = work.tile([128, 512], F32, name="krot_f", tag="rot_f")
                    rope_block(ps_k[t][:], cos8[:, t, :], sin8[:, t, :], krot_f)
                    nc.sync.dma_start(
                        kc_r[t * 128:(t + 1) * 128, :, :],
                        krot_f.rearrange("p (g d) -> p g d", d=64),
                    )
                    krot = work.tile([128, 512], BF16, name="krot", tag="rot")
                    nc.vector.tensor_copy(krot[:], krot_f[:])
                    ps_trk = psT1.tile([64, 8, 128], BF16, name=f"ps_trk_{t}",
                                       tag="ps_tr")
                    for gg in range(8):
                        nc.tensor.transpose(
                            ps_trk[:, gg, :],
                            krot[:, gg * 64:(gg + 1) * 64],
                            ident[:],
                        )
                    nc.vector.tensor_copy(
                        kT[:, :, t * 128:(t + 1) * 128], ps_trk[:]
                    )

                # ---- v ----
                wv_t = wpool.tile([128, KT, 512], BF16, name="wv", tag="wtile")
                nc.sync.dma_start(wv_t[:], wv_d.rearrange("(t p) n -> p t n", p=128))
                ps_v = [psP.tile([128, 512], F32, name=f"ps_v_{t}", tag=f"ps_q{t}")
                        for t in range(TT)]
                for kt in range(KT):
                    for t in range(TT):
                        nc.tensor.matmul(
                            ps_v[t][:], xt[:, kt, t * 128:(t + 1) * 128], wv_t[:, kt, :],
                            start=(kt == 0), stop=(kt == KT - 1),
                        )
                vc_r = vc_d.rearrange("g s d -> s g d")
                for t in range(TT):
                    v_f = work.tile([128, 512], F32, name="v_f", tag="rot_f")
                    nc.vector.tensor_copy(v_f[:], ps_v[t][:])
                    nc.sync.dma_start(
                        vc_r[t * 128:(t + 1) * 128, :, :],
                        v_f.rearrange("p (g d) -> p g d", d=64),
                    )
                    nc.vector.tensor_copy(v_sb[:, t, :], v_f[:])

            # ================= attention per head =============================
            with (
                tc.tile_pool(name="psS", bufs=2, space="PSUM") as psS,
                tc.tile_pool(name="psPT", bufs=2, space="PSUM") as psPT,
                tc.tile_pool(name="psO", bufs=2, space="PSUM") as psO,
            ):
                for h in range(N_HEADS):
                    g = h // GROUPS
                    hp = h // 2
                    ho = (h % 2) * 64
                    psT = psPT.tile([128, TT, 512], BF16, name=f"psT_{h}", tag="psT")
                    for i in range(TT):
                        wdt = (i + 1) * 128
                        ps_s = psS.tile([128, 512], F32, name=f"ps_s_{h}_{i}",
                                        tag="ps_s")
                        nc.tensor.matmul(
                            ps_s[:, :wdt],
                            qT[:, h, i * 128:(i + 1) * 128],
                            kT[:, g, 0:wdt],
                            start=True, stop=True,
                        )
                        nc.vector.tensor_tensor(
                            ps_s[:, wdt - 128:wdt], ps_s[:, wdt - 128:wdt],
                            dmask[:], op=AX.add,
                        )
                        probs = probs_pool.tile([128, 512], BF16,
                                                name=f"probs_{h}_{i}", tag="probs")
                        sums = stat.tile([128, 1], F32, name=f"sums_{h}_{i}",
                                         tag="sums")
                        nc.scalar.activation(
                            probs[:, :wdt], ps_s[:, :wdt],
                            mybir.ActivationFunctionType.Exp,
                            accum_out=sums[:],
                        )
                        rec = stat.tile([128, 1], F32, name=f"rec_{h}_{i}", tag="rec")
                        nc.vector.reciprocal(rec[:], sums[:])
                        nc.vector.tensor_scalar_mul(probs[:, :wdt], probs[:, :wdt],
                                                    rec[:])
                        for j in range(i + 1):
                            nc.tensor.transpose(
                                psT[:, j, i * 128:(i + 1) * 128],
                                probs[:, j * 128:(j + 1) * 128],
                                ident[:],
                            )
                    pT_sb = []
                    for j in range(TT):
                        pt = pT_pool.tile([128, 512], BF16, name=f"pT_{h}_{j}",
                                          tag="pT")
                        nc.scalar.copy(pt[:, j * 128:], psT[:, j, j * 128:])
                        pT_sb.append(pt)
                    ps_o = psO.tile([64, 512], F32, name=f"ps_o_{h}", tag="ps_o")
                    for j in range(TT):
                        nc.tensor.matmul(
                            ps_o[:, j * 128:],
                            v_sb[:, j, g * 64:(g + 1) * 64],
                            pT_sb[j][:, j * 128:],
                            start=(j == 0), stop=(j == TT - 1),
                        )
                    nc.scalar.copy(aoT[ho:ho + 64, hp, :], ps_o[:])

            # ================= o_proj =========================================
            with tc.tile_pool(name="psF", bufs=1, space="PSUM") as psF:
                for o in range(4):
                    wo_t = wpool.tile([128, KT, 512], BF16, name=f"wo_{o}", tag="wtile")
                    nc.sync.dma_start(
                        wo_t[:],
                        wo_d[:, o * 512:(o + 1) * 512].rearrange("(t p) n -> p t n", p=128),
                    )
                    ps_out = [psF.tile([128, 512], F32, name=f"ps_out_{o}_{t}",
                                       tag=f"ps_out{t}") for t in range(TT)]
                    for kt in range(KT):
                        for t in range(TT):
                            nc.tensor.matmul(
                                ps_out[t][:], aoT[:, kt, t * 128:(t + 1) * 128],
                                wo_t[:, kt, :],
                                start=(kt == 0), stop=(kt == KT - 1),
                            )
                    for t in range(TT):
                        ot = work.tile([128, 512], F32, name="out_sb", tag="out_sb")
                        nc.vector.tensor_copy(ot[:], ps_out[t][:])
                        nc.sync.dma_start(
                            out_d[t * 128:(t + 1) * 128, o * 512:(o + 1) * 512], ot[:]
                        )

    return nc


_nc_cache = [None]


def _rope_tables():
    inv_freq = 1.0 / (ROPE_BASE ** (np.arange(0, HEAD_DIM, 2, dtype=np.float32) / HEAD_DIM))
    pos = np.arange(S, dtype=np.float32)
    freqs = np.outer(pos, inv_freq)
    emb = np.concatenate([freqs, freqs], axis=-1)  # [S, D]
    return np.cos(emb).astype(np.float32), np.sin(emb).astype(np.float32)


def prepare_in_maps(x, Wq, Wk, Wv, Wo):
    scale = np.float32(HEAD_DIM ** -0.5)
    cos, sin = _rope_tables()
    cos8 = np.ascontiguousarray(np.tile(cos, (1, N_KV_HEADS)))   # [S, 512]
    sin8 = np.ascontiguousarray(np.tile(sin, (1, N_KV_HEADS)))
    ident = np.eye(128, dtype=np.float32).astype(BF)
    dmask = np.triu(np.full((128, 128), -1e30, dtype=np.float32), k=1)
    wq_s = np.ascontiguousarray((Wq.astype(np.float32) * scale).astype(BF))
    wk = np.ascontiguousarray(Wk.astype(np.float32).astype(BF))
    wv = np.ascontiguousarray(Wv.astype(np.float32).astype(BF))
    wo = np.ascontiguousarray(Wo.astype(np.float32).astype(BF))
    in_maps = []
    for b in range(B):
        in_maps.append({
            "xt": np.ascontiguousarray(x[b].T.astype(np.float32)).astype(BF),
            "wq": wq_s, "wk": wk, "wv": wv, "wo": wo,
            "cos8": cos8, "sin8": sin8, "ident": ident, "dmask": dmask,
        })
    return in_maps


def run(x, Wq, Wk, Wv, Wo, trace=False, **spmd_kwargs):
    if _nc_cache[0] is None:
        _nc_cache[0] = build_nc()
    nc = _nc_cache[0]
    in_maps = prepare_in_maps(x, Wq, Wk, Wv, Wo)
    res = run_bass_kernel_spmd(nc, in_maps, core_ids=list(range(B)), trace=trace,
                               **spmd_kwargs)
    out = np.stack([res.results[b]["out"] for b in range(B)])       # [B, S, H]
    kc = np.stack([res.results[b]["kc"] for b in range(B)])         # [B, Hkv, S, D]
    vc = np.stack([res.results[b]["vc"] for b in range(B)])
    return (out, kc, vc), res


def kernel(x, Wq, Wk, Wv, Wo):
    (out, kc, vc), _ = run(np.asarray(x), np.asarray(Wq), np.asarray(Wk),
                           np.asarray(Wv), np.asarray(Wo), trace=False)
    return out.astype(np.float32), kc.astype(np.float32), vc.astype(np.float32)
